# revision 53
# baseline (speedup 1.0000x reference)
"""Fused DDiT transformer block (causal) on 8 TRN2 NeuronCores.

Sharding: attention is head-parallel (2 heads/core, 16 total) with QKV
column-sliced per core; an AllToAll then re-shards from head-split to
token-split, and out-proj + MLP run token-parallel (512 tokens/core).
LayerNorm gains are folded into the following matmul weights on the host;
LN centering is folded into the matmuls via an appended K=1 rank-1 update
(-mu[t] * rowsum_w[e]) and the 1/std factor is folded into the RoPE tables
(q,k), a PSUM-eviction multiply (v), or a broadcast multiply (MLP).
Compute dtype bf16 (fp32 accumulation); the residual stream stays fp32.
"""
import sys

for _p in ("/opt/trn_rl_repo",):
    if _p not in sys.path:
        sys.path.append(_p)

import numpy as np
import ml_dtypes

import concourse.bass as bass
import concourse.tile as tile
import concourse.mybir as mybir
from concourse.bass_utils import run_bass_kernel_spmd
from concourse.masks import make_identity

bf16 = mybir.dt.bfloat16
f8 = mybir.dt.float8e4
f32 = mybir.dt.float32
AF = mybir.ActivationFunctionType
OP = mybir.AluOpType

N_CORES = 8
B, S, D = 2, 2048, 1024
T = B * S            # 4096 tokens total
NH, HD = 16, 64      # heads, head dim
HPC = NH // N_CORES  # 2 heads per core
TOK = T // N_CORES   # 512 tokens per core in the token-split phase
NT = T // 128        # 32 token tiles of 128
NCH = T // 512       # 8 chunks of 512 tokens
LN_EPS = 1e-5

# ---------------------------------------------------------------------------
# Sync legalizer: this walrus build accepts only ONE sync wait and ONE sync
# update per TPB instruction. Move extras onto same-engine NoOps (engines
# complete instructions in program order, so semantics are preserved).
# ---------------------------------------------------------------------------
_uid = [0]


def _legalize_sync(nc):
    for f in nc.m.functions:
        for bb in f.blocks:
            out = []
            changed = False
            for inst in bb.instructions:
                si = inst.sync_info
                if si is None:
                    out.append(inst)
                    continue
                waits = list(si.on_wait) if si.on_wait else []
                updates = list(si.on_update) if si.on_update else []
                if len(waits) <= 1 and len(updates) <= 1:
                    out.append(inst)
                    continue
                changed = True
                for w in waits[:-1]:
                    _uid[0] += 1
                    nop = mybir.InstNoOp(name=f"syncw-{_uid[0]}", ins=[], outs=[])
                    nop.engine = inst.engine
                    nop.sync_info = mybir.SyncInfo(on_wait=[w], on_update=[])
                    out.append(nop)
                inst.sync_info = mybir.SyncInfo(
                    on_wait=waits[-1:], on_update=updates[:1]
                )
                out.append(inst)
                for u in updates[1:]:
                    _uid[0] += 1
                    nop = mybir.InstNoOp(name=f"syncu-{_uid[0]}", ins=[], outs=[])
                    nop.engine = inst.engine
                    nop.sync_info = mybir.SyncInfo(on_wait=[], on_update=[u])
                    out.append(nop)
            if changed:
                bb.instructions = out
    return nc


# ---------------------------------------------------------------------------
# Kernel graph
# ---------------------------------------------------------------------------
def _build():
    nc = bass.Bass()

    # -- external inputs (per core)
    xT_blk = nc.dram_tensor("xT_blk", (NCH, 128, 8, 512), bf16, kind="ExternalInput")
    xT_f8 = nc.dram_tensor("xT_f8", (NCH, 128, 8, 512), f8, kind="ExternalInput")
    xT_own = nc.dram_tensor("xT_own", (D, TOK), bf16, kind="ExternalInput")
    wqkv_blk = nc.dram_tensor("wqkv_blk", (3, 128, 8, 128), bf16, kind="ExternalInput")
    wqkv_rs = nc.dram_tensor("wqkv_rs", (3, 1, 128), bf16, kind="ExternalInput")
    tab = nc.dram_tensor("tab", (2, 128, T), bf16, kind="ExternalInput")  # cos, sin(signed)
    wout_blk = nc.dram_tensor("wout_blk", (8, 128, 8, 128), bf16, kind="ExternalInput")
    w1_blk = nc.dram_tensor("w1_blk", (32, 128, 8, 128), bf16, kind="ExternalInput")
    w1_rs = nc.dram_tensor("w1_rs", (32, 1, 128), bf16, kind="ExternalInput")
    b1_t = nc.dram_tensor("b1_t", (32, 128, 1), f32, kind="ExternalInput")
    w2_blk = nc.dram_tensor("w2_blk", (8, 128, 32, 128), bf16, kind="ExternalInput")
    b2_t = nc.dram_tensor("b2_t", (8, 128, 1), f32, kind="ExternalInput")
    out_d = nc.dram_tensor("out", (D, TOK), f32, kind="ExternalOutput")

    # -- internal DRAM (one AllToAll buffer pair per head slot)
    # fp8 payload: rows 0:64 = unnormalized o (fp8), rows 64:66 = the bf16
    # softmax-reciprocal row shipped as raw bytes; normalization happens
    # token-split after the AllToAll.
    cc_in_h = [nc.dram_tensor(f"cc_in{h}", (N_CORES, 66, TOK), f8, kind="Internal")
               for h in range(2)]
    cc_out_h = [nc.dram_tensor(f"cc_out{h}", (N_CORES, 66, TOK), f8, kind="Internal")
                for h in range(2)]

    with tile.TileContext(nc) as tc, \
         nc.allow_low_precision(reason="bf16 block compute"):
        with tc.tile_pool(name="const", bufs=1) as pconst, \
             tc.tile_pool(name="persist", bufs=1) as pper, \
             tc.tile_pool(name="stream", bufs=2) as pstream, \
             tc.tile_pool(name="big2", bufs=2) as pbig2, \
             tc.tile_pool(name="work", bufs=3) as pwork:
            ident_bf = pconst.tile([128, 128], bf16)
            make_identity(nc, ident_bf)
            # additive causal mask: 0 where q_local - k_local = f - p >= 0,
            # else -50 (accumulated into scores via ident matmul; exp->~0)
            ones_row = pconst.tile([1, 128], bf16)
            nc.vector.memset(ones_row, 1.0)
            ones_col = pconst.tile([128, 1], bf16)
            nc.vector.memset(ones_col, 1.0)
            eps_col = pconst.tile([128, 1], f32)
            nc.vector.memset(eps_col, LN_EPS)
            ones2_f8 = pconst.tile([128, 2, 128], f8)
            nc.vector.memset(ones2_f8, 1.0)
            # additive causal mask for diagonal tiles: out[p,f] += maskT[f,p],
            # maskT[c,m] = -50 where m - c - 1 >= 0 (strict upper tri of the
            # score tile after the ident-matmul transpose), else 0
            ident_f8 = pconst.tile([128, 128], f8)
            nc.vector.tensor_copy(out=ident_f8, in_=ident_bf)
            # 0/1 causal keep-mask (keep where q_local >= kv_local)
            mask01_bf = pconst.tile([128, 128], bf16)
            nc.vector.memset(mask01_bf, 1.0)
            nc.gpsimd.affine_select(out=mask01_bf, in_=mask01_bf,
                                    pattern=[[1, 128]], compare_op=OP.is_ge,
                                    fill=0.0, base=0, channel_multiplier=-1)
            mask01_f8 = pconst.tile([128, 128], f8)
            nc.vector.tensor_copy(out=mask01_f8, in_=mask01_bf)

            # =============================================================
            # Phase A/B fused: per-chunk LN1 stats on PE + QKV + RoPE + V
            # =============================================================
            attn_pool_cm = tc.tile_pool(name="attn", bufs=1)
            pattn = attn_pool_cm.__enter__()
            negmu_row = pattn.tile([1, T], bf16)
            rstd_row = pattn.tile([1, T], bf16)
            rstd_sb = pattn.tile([128, T], bf16)

            # chunk-0 prefetches first: the LN1 stats matmuls are the
            # first PE work and need the fp8 x before any weights
            xf8_pre = pattn.tile([128, 8, 512], f8, tag="xf8", bufs=3)
            nc.sync.dma_start(out=xf8_pre, in_=xT_f8[0])
            xf8_pre1 = pattn.tile([128, 8, 512], f8, tag="xf8", bufs=3,
                                  name="xf8_1p")
            nc.sync.dma_start(out=xf8_pre1, in_=xT_f8[1])
            xrt_pre = pbig2.tile([128, 8, 512], bf16, tag="xTr")
            for _pc in range(4):
                nc.sync.dma_start(out=xrt_pre[:, 2 * _pc:2 * _pc + 2, :],
                                  in_=xT_blk[0, :, 2 * _pc:2 * _pc + 2, :])

            # persistent QKV weight tiles (one packed tile per m)
            wq_sb = {}
            for m in range(3):
                w = pconst.tile([128, 8, 128], bf16, name=f"wqkv_{m}", tag=f"wqkv_{m}")
                nc.sync.dma_start(out=w, in_=wqkv_blk[m])
                wq_sb[m] = w
            rs_sb = {}
            for m in range(3):
                r = pconst.tile([1, 128], bf16, name=f"wqkvrs_{m}", tag=f"wqkvrs_{m}")
                nc.sync.dma_start(out=r, in_=wqkv_rs[m])
                rs_sb[m] = r


            # =============================================================
            # Phase B: QKV projection + RoPE + V transpose
            # =============================================================
            qT_sb = pattn.tile([128, T], bf16)
            # per-head K tables zero-padded to the full 128 partitions so the
            # score matmuls run with a full-width stationary tile
            kTp = [pattn.tile([128, T], bf16, name=f"kTp{h}", tag=f"kTp{h}")
                   for h in range(2)]
            nc.vector.memset(kTp[0][64:128, :], 0.0)
            nc.gpsimd.memset(kTp[1][0:64, :], 0.0)
            v_all = pattn.tile([128, NT, 130], bf16)

            with tc.tile_pool(name="psQKV", bufs=3, space="PSUM") as psQ, \
                 tc.tile_pool(name="psVT", bufs=1, space="PSUM") as psVT, \
                 tc.tile_pool(name="psST", bufs=1, space="PSUM") as psST:
                def emit_stats(ch, xf8):
                    # LN1 stats for chunk ch (emitted one chunk ahead so the
                    # slow recip row-math is off the PE's critical path).
                    # fp8 DoubleRow halves the PE stream count; fp8 rounding
                    # is benign for sums (unbiased, averages out over 1024).
                    sl_ = slice(ch * 512, (ch + 1) * 512)
                    xsq = pattn.tile([128, 8, 512], f8, tag="xsq", bufs=1)
                    nc.scalar.activation(out=xsq, in_=xf8, func=AF.Square)
                    ps_mu2 = psST.tile([128, 512], f32, tag="mu")
                    ps_sq2 = psST.tile([128, 512], f32, tag="sq")
                    ps_mu, ps_sq = ps_mu2[0:1, :], ps_sq2[0:1, :]
                    for j in range(4):
                        nc.tensor.matmul(ps_mu2, ones2_f8, xf8[:, 2 * j:2 * j + 2, :],
                                         start=(j == 0), stop=(j == 3),
                                         perf_mode=mybir.MatmulPerfMode.DoubleRow)
                    for j in range(4):
                        nc.tensor.matmul(ps_sq2, ones2_f8, xsq[:, 2 * j:2 * j + 2, :],
                                         start=(j == 0), stop=(j == 3),
                                         perf_mode=mybir.MatmulPerfMode.DoubleRow)
                    nc.vector.tensor_scalar_mul(out=negmu_row[0:1, sl_],
                                                in0=ps_mu, scalar1=-1.0 / D)
                    mus_c = pwork.tile([1, 512], f32, tag="mus1_r", bufs=2)
                    nc.vector.tensor_mul(out=mus_c, in0=negmu_row[0:1, sl_],
                                         in1=negmu_row[0:1, sl_])
                    var_c = pwork.tile([1, 512], f32, tag="var1_r", bufs=2)
                    nc.vector.scalar_tensor_tensor(
                        out=var_c, in0=ps_sq, scalar=1.0 / D, in1=mus_c,
                        op0=OP.mult, op1=OP.subtract)
                    sd_c = pwork.tile([1, 512], f32, tag="sd1_r", bufs=2)
                    nc.scalar.activation(out=sd_c, in_=var_c, func=AF.Sqrt,
                                         bias=eps_col[0:1, :])
                    rstd_c = pwork.tile([1, 512], f32, tag="rowf32", bufs=2)
                    nc.vector.reciprocal(out=rstd_c, in_=sd_c)
                    nc.vector.tensor_copy(out=rstd_row[0:1, sl_], in_=rstd_c)

                # fp8 x is prefetched two chunks ahead so the stats
                # matmuls never wait on their DMA
                xf8_tiles = {0: xf8_pre, 1: xf8_pre1}
                # rope tables after the chunk-0/1 critical prefetches
                # (first consumed at chunk-0's rope, ~20us in)
                tabs = []
                for ti in range(2):
                    raw = pattn.tile([128, T], bf16, name=f"tab{ti}", tag=f"tab{ti}")
                    nc.sync.dma_start(out=raw, in_=tab[ti])
                    tabs.append(raw)
                tab_c, tab_s = tabs
                emit_stats(0, xf8_pre)
                xrt = xrt_pre
                for ch in range(NCH):
                    sl = slice(ch * 512, (ch + 1) * 512)
                    if ch + 2 < NCH:
                        xf8_tiles[ch + 2] = pattn.tile([128, 8, 512], f8,
                                                       tag="xf8", bufs=3,
                                                       name=f"xf8_{ch + 2}")
                        nc.sync.dma_start(out=xf8_tiles[ch + 2], in_=xT_f8[ch + 2])
                    if ch + 1 < NCH:
                        xrt_next = pbig2.tile([128, 8, 512], bf16, tag="xTr")
                        nc.sync.dma_start(out=xrt_next, in_=xT_blk[ch + 1])
                        emit_stats(ch + 1, xf8_tiles[ch + 1])
                    for m in range(3):
                        ps = psQ.tile([128, 512], f32, tag="qkv")
                        for kk in range(8):
                            nc.tensor.matmul(ps, wq_sb[m][:, kk, :], xrt[:, kk, :],
                                             start=(kk == 0), stop=False)
                        if m == 0:
                            # broadcast rstd AFTER m0's contraction so the PE
                            # isn't queued behind the slow recip row-math;
                            # fold rstd into the rope tables for this chunk
                            ps_b = psQ.tile([128, 512], f32, tag="bc", bufs=1)
                            nc.tensor.matmul(ps_b, ones_row[0:1, 0:128],
                                             rstd_row[0:1, sl],
                                             start=True, stop=True)
                            nc.scalar.activation(out=rstd_sb[:, sl], in_=ps_b,
                                                 func=AF.Copy)
                            # gpsimd: vector is co-critical with the PE in
                            # this phase (84.7 vs 87.2us in-window)
                            nc.gpsimd.tensor_mul(out=tab_c[:, sl], in0=tab_c[:, sl],
                                                 in1=rstd_sb[:, sl])
                            nc.gpsimd.tensor_mul(out=tab_s[:, sl], in0=tab_s[:, sl],
                                                 in1=rstd_sb[:, sl])
                        nc.tensor.matmul(ps, rs_sb[m], negmu_row[0:1, sl],
                                         start=False, stop=True)
                        if m < 2:  # q or k: rope
                            tc_t = pwork.tile([128, 512], bf16, tag="ropec", bufs=2)
                            nc.scalar.activation(out=tc_t, in_=ps, func=AF.Copy)
                            tsw = pwork.tile([128, 512], bf16, tag="ropesw", bufs=2)
                            for h in range(2):
                                for a2 in range(2):
                                    nc.sync.dma_start(
                                        out=tsw[h * 64 + a2 * 32:h * 64 + a2 * 32 + 32, :],
                                        in_=tc_t[h * 64 + (1 - a2) * 32:h * 64 + (1 - a2) * 32 + 32, :])
                            tabc, tabs_ = tab_c, tab_s
                            t1 = pwork.tile([128, 512], bf16, tag="ropet1", bufs=2)
                            nc.vector.tensor_mul(out=t1, in0=tc_t, in1=tabc[:, sl])
                            t2 = pwork.tile([128, 512], bf16, tag="ropet2", bufs=2)
                            nc.vector.tensor_mul(out=t2, in0=tsw, in1=tabs_[:, sl])
                            if m == 0:
                                nc.vector.tensor_add(out=qT_sb[:, sl], in0=t1, in1=t2)
                            else:  # k: split into the two padded head tables
                                nc.vector.tensor_add(out=kTp[0][0:64, sl],
                                                     in0=t1[0:64, :], in1=t2[0:64, :])
                                nc.vector.tensor_add(out=kTp[1][64:128, sl],
                                                     in0=t1[64:128, :], in1=t2[64:128, :])
                        else:  # v: scale by rstd, transpose to [t, e] tiles
                            vt = pwork.tile([128, 512], bf16, tag="vtmp")
                            nc.vector.tensor_mul(out=vt, in0=ps, in1=rstd_sb[:, sl])
                            for j in range(4):
                                g = ch * 4 + j
                                pst = psVT.tile([128, 128], bf16, tag="vtr")
                                nc.tensor.transpose(out=pst, in_=vt[:, j * 128:(j + 1) * 128],
                                                    identity=ident_bf)
                                nc.vector.tensor_copy(out=v_all[:, g, 0:64], in_=pst[:, 0:64])
                                nc.vector.tensor_copy(out=v_all[:, g, 65:129], in_=pst[:, 64:128])
                                nc.vector.memset(v_all[:, g, 64:65], 1.0)
                                nc.vector.memset(v_all[:, g, 129:130], 1.0)
                    if ch + 1 < NCH:
                        xrt = xrt_next

            # =============================================================
            # Phase C: causal attention per (batch, head), transposed layout
            # =============================================================
            with tc.tile_pool(name="psSC", bufs=4, space="PSUM") as psSC, \
                 tc.tile_pool(name="psO", bufs=4, space="PSUM") as psO:
                for h in range(2):
                    hsl = slice(h * 64, (h + 1) * 64)
                    for qc in range(4):
                        nkt = 4 * (qc + 1)
                        ps_o = [psO.tile([65, 512], f32, tag="o",
                                         name=f"o_{h}_{qc}_{bb}")
                                for bb in range(2)]
                        pend = []  # software pipeline: AV lags scores by one kt
                        for kt in range(nkt):
                            cur = []
                            for b in range(2):
                                diag = kt >= 4 * qc
                                off = kt * 128 - qc * 512 if diag else 0
                                qsl = slice(b * 2048 + qc * 512 + off,
                                            b * 2048 + (qc + 1) * 512)
                                ksl = slice(b * 2048 + kt * 128,
                                            b * 2048 + (kt + 1) * 128)
                                ps_s = psSC.tile([128, 512], f32, tag="sc")
                                nc.tensor.matmul(ps_s[:, off:], kTp[h][:, ksl],
                                                 qT_sb[:, qsl],
                                                 start=True, stop=True)
                                p_t = pwork.tile([128, 512], bf16, tag="p", bufs=10)
                                nc.scalar.activation(out=p_t[:, off:],
                                                     in_=ps_s[:, off:], func=AF.Exp)
                                if diag:  # causal mask on the idle gpsimd
                                    nc.gpsimd.affine_select(
                                        out=p_t[:, off:off + 128],
                                        in_=p_t[:, off:off + 128],
                                        pattern=[[1, 128]], compare_op=OP.is_ge,
                                        fill=0.0, base=0, channel_multiplier=-1)
                                cur.append((b, kt, p_t, off))
                            for (b2, kt2, p2, off2) in pend:
                                g = b2 * 16 + kt2
                                nc.tensor.matmul(ps_o[b2][:, off2:],
                                                 v_all[:, g, h * 65:(h + 1) * 65],
                                                 p2[:, off2:],
                                                 start=(kt2 == 0),
                                                 stop=(kt2 == nkt - 1))
                            pend = cur
                        for (b2, kt2, p2, off2) in pend:  # drain pipeline
                            g = b2 * 16 + kt2
                            nc.tensor.matmul(ps_o[b2][:, off2:],
                                             v_all[:, g, h * 65:(h + 1) * 65],
                                             p2[:, off2:],
                                             start=(kt2 == 0),
                                             stop=(kt2 == nkt - 1))
                        for b in range(2):
                            # ship unnormalized o plus the reciprocal row;
                            # the divide happens token-split after the A2A
                            o_t = pwork.tile([64, 512], f8, tag="o_t")
                            # vector, not scalar: the exps saturate scalar in
                            # this phase, and a [64,512] copy is cheaper on DVE
                            nc.vector.tensor_copy(out=o_t, in_=ps_o[b][0:64, :])
                            # evict the denominator row cheaply so the PSUM
                            # bank isn't held hostage by the slow reciprocal
                            den_sb = pwork.tile([1, 512], f32, tag="densb", bufs=2)
                            nc.vector.tensor_copy(out=den_sb, in_=ps_o[b][64:65, :])
                            rec_f = pwork.tile([1, 512], f32, tag="rowf32", bufs=2)
                            nc.vector.reciprocal(out=rec_f, in_=den_sb)
                            rec_b = pwork.tile([1, 512], bf16, tag="rec_b", bufs=2)
                            nc.vector.tensor_copy(out=rec_b, in_=rec_f)
                            j = b * 4 + qc
                            nc.sync.dma_start(out=cc_in_h[h][j, 0:64, :], in_=o_t)
                            nc.sync.dma_start(
                                out=cc_in_h[h][j, 64:66, :].bitcast(bf16),
                                in_=rec_b)
                    # fire this head's AllToAll; h=1 compute overlaps h=0's
                    nc.gpsimd.collective_compute(
                        "AllToAll", OP.bypass, ins=[cc_in_h[h][:, :, :]],
                        outs=[cc_out_h[h][:, :, :]],
                        replica_groups=[list(range(N_CORES))])

            attn_pool_cm.__exit__(None, None, None)
            mlp_pool_cm = tc.tile_pool(name="mlp", bufs=1)
            pmlp = mlp_pool_cm.__enter__()

            # =============================================================
            # Phase D: out-proj + LN2 (A2As already fired per head above)
            # =============================================================
            o_own = [pmlp.tile([128, 512], f8, name=f"oo_{kk}", tag=f"oo_{kk}") for kk in range(8)]
            # normalized bf16 o tiles reuse the u_g buffers (disjoint lifetime)
            o_bf = [pmlp.tile([128, 512], bf16, name=f"ob_{kk}", tag=f"ug_{kk}") for kk in range(8)]
            xT_o = [pmlp.tile([128, 512], bf16, name=f"xo_{kk}", tag=f"xo_{kk}") for kk in range(8)]
            # softmax recs: index s*8+kk; h0/h1 share buffers (tag, bufs=1)
            rec16 = [pmlp.tile([1, TOK], bf16, name=f"rc_{i}", tag=f"rc_{i % 8}", bufs=1)
                     for i in range(16)]
            for kk in range(8):  # h0 halves first: ready right after A2A#1
                nc.sync.dma_start(out=o_own[kk][0:64, :], in_=cc_out_h[0][kk][0:64, :])
                nc.sync.dma_start(out=rec16[kk], in_=cc_out_h[0][kk, 64:66, :].bitcast(bf16))
            for kk in range(8):
                nc.sync.dma_start(out=o_own[kk][64:128, :], in_=cc_out_h[1][kk][0:64, :])
                nc.sync.dma_start(out=xT_o[kk], in_=xT_own[kk * 128:(kk + 1) * 128, :])
                nc.sync.dma_start(out=rec16[8 + kk], in_=cc_out_h[1][kk, 64:66, :].bitcast(bf16))

            xa = [pmlp.tile([128, 512], f32, name=f"xa_{m}", tag=f"xa_{m}") for m in range(8)]
            xab = [pmlp.tile([128, 512], bf16, name=f"xab_{m}", tag=f"xab_{m}") for m in range(8)]
            with tc.tile_pool(name="psOP", bufs=4, space="PSUM") as psOP, \
                 tc.tile_pool(name="psMU", bufs=1, space="PSUM") as psMU, \
                 tc.tile_pool(name="psSQ", bufs=1, space="PSUM") as psSQ, \
                 tc.tile_pool(name="psRB", bufs=2, space="PSUM") as psRB:
                ps_mu = psMU.tile([1, 512], f32)
                ps_sq = psSQ.tile([1, 512], f32)
                # normalize h0 halves (overlaps A2A#2)
                for kk in range(8):
                    ps_nr = psRB.tile([64, 512], f32, tag="nr")
                    nc.tensor.matmul(ps_nr, ones_row[0:1, 0:64],
                                     rec16[kk], start=True, stop=True)
                    nc.vector.tensor_mul(out=o_bf[kk][0:64, :],
                                         in0=o_own[kk][0:64, :], in1=ps_nr)
                # m=0..3: h0-half contraction prepass — overlaps A2A#2
                ps_hold = {}
                for m in range(4):
                    ps = psOP.tile([128, 512], f32, tag="op", name=f"op_{m}")
                    w = pstream.tile([64, 8, 128], bf16, tag="wo_h", bufs=2)
                    nc.sync.dma_start(out=w, in_=wout_blk[m, 0:64, :, :])
                    for kk in range(8):
                        nc.tensor.matmul(ps, w[0:64, kk, :], o_bf[kk][0:64, :],
                                         start=(kk == 0), stop=False)
                    ps_hold[m] = ps
                # normalize h1 halves as soon as A2A#2 lands
                for kk in range(8):
                    ps_nr = psRB.tile([64, 512], f32, tag="nr")
                    nc.tensor.matmul(ps_nr, ones_row[0:1, 0:64],
                                     rec16[8 + kk], start=True, stop=True)
                    nc.vector.tensor_mul(out=o_bf[kk][64:128, :],
                                         in0=o_own[kk][64:128, :], in1=ps_nr)
                for m in range(8):
                    if m < 4:
                        ps = ps_hold[m]
                        w = pstream.tile([128, 8, 128], bf16, tag="wo_st", bufs=2)
                        nc.sync.dma_start(out=w[64:128, :, :],
                                          in_=wout_blk[m, 64:128, :, :])
                        for kk in range(8):
                            nc.tensor.matmul(ps, w[64:128, kk, :],
                                             o_bf[kk][64:128, :],
                                             start=False, stop=(kk == 7))
                    else:
                        ps = psOP.tile([128, 512], f32, tag="op", name=f"op_{m}")
                        w = pstream.tile([128, 8, 128], bf16, tag="wo_st", bufs=2)
                        nc.sync.dma_start(out=w[:, 0:4, :], in_=wout_blk[m, :, 0:4, :])
                        nc.sync.dma_start(out=w[:, 4:8, :], in_=wout_blk[m, :, 4:8, :])
                        for kk in range(8):
                            nc.tensor.matmul(ps, w[:, kk, :], o_bf[kk],
                                             start=(kk == 0), stop=(kk == 7))
                    nc.vector.tensor_add(out=xa[m], in0=ps, in1=xT_o[m])
                    nc.vector.tensor_copy(out=xab[m], in_=xa[m])
                    b2 = pwork.tile([128, 1], f32, tag="b2_st")
                    nc.sync.dma_start(out=b2, in_=b2_t[m])
                    nc.scalar.activation(out=xa[m], in_=xa[m], func=AF.Identity,
                                         bias=b2)
                    sq = pwork.tile([128, 512], bf16, tag="sq", bufs=2)
                    nc.scalar.activation(out=sq, in_=xab[m], func=AF.Square)
                    nc.tensor.matmul(ps_mu, ones_col, xab[m],
                                     start=(m == 0), stop=(m == 7))
                    nc.tensor.matmul(ps_sq, ones_col, sq,
                                     start=(m == 0), stop=(m == 7))

                # LN2 row stats: mu = sum/1024, var = sqsum/1024 - mu^2
                negmu2 = pmlp.tile([1, 512], bf16)
                nc.vector.tensor_scalar_mul(out=negmu2, in0=ps_mu, scalar1=-1.0 / D)
                mus_r = pwork.tile([1, 512], f32, tag="mus_r", bufs=1)
                nc.vector.tensor_mul(out=mus_r, in0=negmu2, in1=negmu2)
                var_r = pwork.tile([1, 512], f32, tag="var_r", bufs=1)
                nc.vector.scalar_tensor_tensor(
                    out=var_r, in0=ps_sq, scalar=1.0 / D, in1=mus_r,
                    op0=OP.mult, op1=OP.subtract)
                sd_r = pwork.tile([1, 512], f32, tag="sd_r", bufs=1)
                nc.scalar.activation(out=sd_r, in_=var_r, func=AF.Sqrt, bias=eps_col[0:1, :])
                rstd2_f = pwork.tile([1, 512], f32, tag="rowf32", bufs=2)
                nc.vector.reciprocal(out=rstd2_f, in_=sd_r)
                rstd2 = pmlp.tile([1, 512], bf16)
                nc.vector.tensor_copy(out=rstd2, in_=rstd2_f)
                rstd2_sb = pmlp.tile([128, 512], bf16)

            # =============================================================
            # Phase E: MLP (token-split, full weights)
            # =============================================================
            u_g = [pmlp.tile([128, 512], bf16, name=f"ug_{m}", tag=f"ug_{m}") for m in range(32)]
            with tc.tile_pool(name="psU", bufs=4, space="PSUM") as psU, \
                 tc.tile_pool(name="psDn", bufs=2, space="PSUM") as psDn:
                negmu2_sb = pmlp.tile([128, 512], bf16)
                for m in range(32):
                    ps = psU.tile([128, 512], f32, tag="u")
                    w = pstream.tile([128, 8, 128], bf16, tag="w1_st", bufs=6)
                    nc.sync.dma_start(out=w[:, 0:4, :], in_=w1_blk[m, :, 0:4, :])
                    nc.sync.dma_start(out=w[:, 4:8, :], in_=w1_blk[m, :, 4:8, :])
                    for kk in range(8):
                        nc.tensor.matmul(ps, w[:, kk, :], xab[kk],
                                         start=(kk == 0),
                                         stop=(m > 0 and kk == 7))
                    if m == 0:
                        # m0 runs the rank-1-corrected path while the LN2
                        # broadcasts land; then xab is normalized in place so
                        # m>=1 needs neither rank-1 nor the eviction multiply
                        ps_rb = psU.tile([128, 512], f32, tag="rb2", bufs=1)
                        nc.tensor.matmul(ps_rb, ones_row[0:1, 0:128], rstd2,
                                         start=True, stop=True)
                        nc.scalar.activation(out=rstd2_sb, in_=ps_rb, func=AF.Copy)
                        ps_nb = psU.tile([128, 512], f32, tag="nb2", bufs=1)
                        nc.tensor.matmul(ps_nb, ones_row[0:1, 0:128], negmu2,
                                         start=True, stop=True)
                        nc.scalar.activation(out=negmu2_sb, in_=ps_nb, func=AF.Copy)
                        for kk in range(8):
                            nc.vector.tensor_add(out=xab[kk], in0=xab[kk],
                                                 in1=negmu2_sb)
                            nc.vector.tensor_mul(out=xab[kk], in0=xab[kk],
                                                 in1=rstd2_sb)
                        r1 = pwork.tile([1, 128], bf16, tag="w1rs_st")
                        nc.sync.dma_start(out=r1, in_=w1_rs[m])
                        nc.tensor.matmul(ps, r1, negmu2, start=False, stop=True)
                        upre = pwork.tile([128, 512], bf16, tag="upre", bufs=2)
                        nc.vector.tensor_mul(out=upre, in0=ps, in1=rstd2_sb)
                        b1 = pwork.tile([128, 1], f32, tag="b1_st")
                        nc.sync.dma_start(out=b1, in_=b1_t[m])
                        nc.scalar.activation(out=u_g[m], in_=upre,
                                             func=AF.Gelu_apprx_tanh, bias=b1)
                    else:
                        b1 = pwork.tile([128, 1], f32, tag="b1_st")
                        nc.sync.dma_start(out=b1, in_=b1_t[m])
                        nc.scalar.activation(out=u_g[m], in_=ps,
                                             func=AF.Gelu_apprx_tanh, bias=b1)
                for m in range(8):
                    ps = psDn.tile([128, 512], f32, tag="dn")
                    w = pstream.tile([128, 32, 128], bf16, tag="w2_st", bufs=2)
                    for q4 in range(4):
                        nc.sync.dma_start(out=w[:, q4 * 8:(q4 + 1) * 8, :],
                                          in_=w2_blk[m, :, q4 * 8:(q4 + 1) * 8, :])
                    for kk in range(32):
                        nc.tensor.matmul(ps, w[:, kk, :], u_g[kk], start=(kk == 0), stop=(kk == 31))
                    ot = pwork.tile([128, 512], f32, tag="ot", bufs=2)
                    nc.vector.tensor_add(out=ot, in0=ps, in1=xa[m])
                    nc.sync.dma_start(out=out_d[m * 128:(m + 1) * 128, :], in_=ot)
            mlp_pool_cm.__exit__(None, None, None)

    _legalize_sync(nc)
    return nc


# ---------------------------------------------------------------------------
# Host-side prep + execution
# ---------------------------------------------------------------------------
_NC_CACHE = {}


def _get_nc():
    if "nc" not in _NC_CACHE:
        _NC_CACHE["nc"] = _build()
    return _NC_CACHE["nc"]


def _bf(a):
    return np.ascontiguousarray(a).astype(ml_dtypes.bfloat16)


def _f32(a):
    return np.ascontiguousarray(a, dtype=np.float32)


def _prep_inputs(x, rot_cos, rot_sin, ln1_w, w_qkv, w_out, ln2_w, w_mlp1,
                 b_mlp1, w_mlp2, b_mlp2):
    x = np.asarray(x, np.float32)
    X = x.reshape(T, D)

    xT = X.T  # (D, T)
    # (ch, p, kk, t): partition row p holds all kk-blocks contiguously
    _xb = np.ascontiguousarray(xT.reshape(8, 128, NCH, 512).transpose(2, 1, 0, 3))
    xT_blk = _xb.astype(ml_dtypes.bfloat16)
    xT_blk_f8 = _xb.astype(ml_dtypes.float8_e4m3)

    # rope tables: (128 rows = 2 heads x [first32|last32]) x T tokens
    cos = np.asarray(rot_cos, np.float32)[0, :, 0, 0, :HD // 2]  # (S, 32)
    sin = np.asarray(rot_sin, np.float32)[0, :, 0, 0, :HD // 2]
    cT = np.concatenate([cos, cos], 1).T          # (64, S)
    sT = np.concatenate([-sin, sin], 1).T         # (64, S) sign-folded
    cT = np.tile(cT, (2, B))                      # (128, T)
    sT = np.tile(sT, (2, B))
    tab = _bf(np.stack([cT, sT]))

    wqkv_eff = np.asarray(w_qkv, np.float32) * np.asarray(ln1_w, np.float32)[None, :]
    w1_eff = np.asarray(w_mlp1, np.float32) * np.asarray(ln2_w, np.float32)[None, :]
    w_out_f = np.asarray(w_out, np.float32)
    w2_f = np.asarray(w_mlp2, np.float32)

    woutT = w_out_f.T  # (d_in=head dims, e)
    wout_blk = _bf(woutT.reshape(8, 128, 8, 128).transpose(2, 1, 0, 3))  # [m, p, kk, e]
    w1T = w1_eff.T     # (D, 4D)
    w1_blk = _bf(w1T.reshape(8, 128, 32, 128).transpose(2, 1, 0, 3))
    w1_rs = _bf(w1_eff.sum(1).reshape(32, 1, 128))
    w2T = w2_f.T       # (4D, D)
    w2_blk = _bf(w2T.reshape(32, 128, 8, 128).transpose(2, 1, 0, 3))
    b1_arr = _f32(np.asarray(b_mlp1, np.float32).reshape(32, 128, 1))
    b2_arr = _f32(np.asarray(b_mlp2, np.float32).reshape(8, 128, 1))

    in_maps = []
    for c in range(N_CORES):
        w_sl = np.concatenate(
            [wqkv_eff[0 * D + 2 * c * HD: 0 * D + 2 * (c + 1) * HD] * 0.125,
             wqkv_eff[1 * D + 2 * c * HD: 1 * D + 2 * (c + 1) * HD],
             wqkv_eff[2 * D + 2 * c * HD: 2 * D + 2 * (c + 1) * HD]], 0)  # (384, D)
        wT_sl = w_sl.T  # (D, 384) -> [m, p, kk, e]
        wqkv_b = _bf(wT_sl.reshape(8, 128, 3, 128).transpose(2, 1, 0, 3))
        wqkv_rsum = _bf(w_sl.sum(1).reshape(3, 1, 128))
        in_maps.append({
            "xT_blk": xT_blk,
            "xT_f8": xT_blk_f8,
            "xT_own": _bf(xT[:, c * TOK:(c + 1) * TOK]),
            "wqkv_blk": wqkv_b,
            "wqkv_rs": wqkv_rsum,
            "tab": tab,
            "wout_blk": wout_blk,
            "w1_blk": w1_blk,
            "w1_rs": w1_rs,
            "b1_t": b1_arr,
            "w2_blk": w2_blk,
            "b2_t": b2_arr,
        })
    return in_maps


def _assemble(results):
    outT = np.concatenate([results[c]["out"] for c in range(N_CORES)], axis=1)
    return np.ascontiguousarray(outT.T.astype(np.float32)).reshape(B, S, D)


def run_spmd(in_maps, **kwargs):
    nc = _get_nc()
    return run_bass_kernel_spmd(nc, in_maps, core_ids=list(range(N_CORES)), **kwargs)


def kernel(x, rot_cos, rot_sin, ln1_w, w_qkv, w_out, ln2_w, w_mlp1, b_mlp1,
           w_mlp2, b_mlp2):
    in_maps = _prep_inputs(x, rot_cos, rot_sin, ln1_w, w_qkv, w_out, ln2_w,
                           w_mlp1, b_mlp1, w_mlp2, b_mlp2)
    res = run_spmd(in_maps)
    return _assemble(res.results)

